# revision 1
# baseline (speedup 1.0000x reference)
"""Dynamic Directional Attention on 8 trn2 NeuronCores (Bass/Tile).

Problem: B=4, L=S=2048, H=8, E=64, f32.
  qt = tanh(q * 1/(std_H(q)+eps) * dw) * dyn     (std over the HEAD dim, ddof=1:
                                                  reference does std(axis=-2) on
                                                  [B,L,H,E], i.e. over H=8)
  kt likewise; scores[b,h,l,s] = qt . kt          (contract E)
  tau[l] = sqrt(var_s(scores[l,:], ddof=1) + eps)
  A = softmax(scale * scores / tau);  out = A @ v  [B,L,H,E]

Sharding: the head-std couples all 8 heads, so shard 8 cores = 4 batches x 2
L-halves. Each core gets q[b, half] = [1024, 512] and the full k/v[b] =
[2048, 512] (replicated across the half-pair), all heads contiguous in the
free dim - clean 2KB-row DMAs, no collectives.

Per-core kernel:
  - transform in natural layout: per l-row, 8-head strided tensor_reduce for
    sum/sumsq -> var -> rstd[l,e]; q*rstd (head-broadcast AP) -> tanh (ACT,
    scale=dw) -> bf16
  - PE-transpose transformed q,k into [e,l] per head-pair (2 heads per 128-row
    transpose)
  - pass 1 per head: S1[l,s] = tq @ tk^T (bf16, K=64) -> PSUM; bn_stats row
    var -> tau -> m[l] = scale*dyn^2/tau[l]   (scores = dyn^2 * S1)
  - fold m into q: qts[e,l] = tq[e,l]*m[l] (m transposed to a row via PE, DRAM
    bounce, broadcast-DMA), then st[s,l] = tk_chunk^T @ qts -> PSUM -> Exp on
    ACT -> A^T bf16. No max-subtraction needed: scaled scores have std 0.125.
  - A@V with V augmented by a ones column: out^T[d,l] accumulates over
    s-chunks; row 64 = softmax denominator. PE-transpose back to [l, 65],
    reciprocal + per-partition scale on DVE -> normalized output.
"""

import os
import sys

for _p in ("/opt/trn_rl_repo", "/root/.axon_site/_ro/trn_rl_repo"):
    if os.path.isdir(_p) and _p not in sys.path:
        sys.path.append(_p)

import numpy as np

import concourse.bass as bass
import concourse.mybir as mybir
import concourse.tile as tile
from concourse import bacc
from concourse.bass_utils import run_bass_kernel_spmd
from concourse.masks import make_identity

F32 = mybir.dt.float32
BF16 = mybir.dt.bfloat16
AF = mybir.ActivationFunctionType

B, L, S, H, E = 4, 2048, 2048, 8, 64
LC = L // 2          # 1024 l-rows per core
D = H * E            # 512 free-dim columns per core (all 8 heads)
P = 128
NLT = LC // P        # 8 l-chunks
NST = S // P         # 16 s-chunks
NLB = 2              # l-blocks of 512 for the st/AV phase
LB = 512
NHP = H // 2         # 4 head-pairs
EPS = 1e-6
SCALE = 1.0 / np.sqrt(E)
UNB_H = float(H) / float(H - 1)  # ddof=1 over heads
UNB_S = float(S) / float(S - 1)  # ddof=1 over score rows

_last_exec_time_ns = None


def _ensure_axon_hooks():
    """Provide antenv.axon_hooks (NTFF profiling hook) if the image lacks it.

    Mirrors trn_agent_boot.trn_boot's ctypes shim against libaxon_pjrt.so.
    Only used when BASS_TRACE is set; harmless otherwise.
    """
    try:
        import antenv.axon_hooks  # noqa: F401

        return
    except ImportError:
        pass
    import contextlib
    import ctypes
    import types

    try:
        import antenv
    except ImportError:
        return

    holder = {"h": None}
    mod = types.ModuleType("antenv.axon_hooks")
    mod.set_axon_ntff_profile_hook = lambda h: holder.__setitem__("h", h)
    mod.get_axon_ntff_profile_hook = lambda: holder["h"]
    sys.modules["antenv.axon_hooks"] = mod
    antenv.axon_hooks = mod

    so_path = "/opt/axon/libaxon_pjrt.so"
    if not os.path.exists(so_path):
        return
    try:
        lib = ctypes.CDLL(so_path)
    except OSError:
        return
    if not hasattr(lib, "axon_start_nrt_profile"):
        return
    lib.axon_start_nrt_profile.argtypes = [
        ctypes.POINTER(ctypes.c_int64),
        ctypes.c_size_t,
    ]
    lib.axon_start_nrt_profile.restype = ctypes.c_int64
    lib.axon_stop_nrt_profile.argtypes = [ctypes.c_char_p]
    lib.axon_stop_nrt_profile.restype = ctypes.c_int64

    @contextlib.contextmanager
    def _hook(output_dir, device_ids):
        import jax

        jax.devices()
        if device_ids:
            ids = (ctypes.c_int64 * len(device_ids))(*device_ids)
            rc = lib.axon_start_nrt_profile(ids, len(device_ids))
        else:
            rc = lib.axon_start_nrt_profile(None, 0)
        if rc != 0:
            raise RuntimeError(f"axon_start_nrt_profile rc={rc}")
        try:
            yield
        finally:
            n = lib.axon_stop_nrt_profile(str(output_dir).encode())
            print(f"profile: {n} file(s) written to {output_dir}", file=sys.stderr)

    holder["h"] = _hook


def _head_bcast(ap_2d, nh=H, ne=E):
    """View a [p, ne] AP as [p, nh, ne] with the head dim broadcast (step 0)."""
    return bass.AP(
        tensor=ap_2d.tensor,
        offset=ap_2d.offset,
        ap=[list(ap_2d.ap[0]), [0, nh], list(ap_2d.ap[1])],
    )


def build_nc():
    nc = bacc.Bacc("TRN2", target_bir_lowering=False, debug=False)
    q_d = nc.dram_tensor("q", [LC, D], F32, kind="ExternalInput")
    k_d = nc.dram_tensor("k", [S, D], F32, kind="ExternalInput")
    v_d = nc.dram_tensor("v", [S, D], F32, kind="ExternalInput")
    dw_d = nc.dram_tensor("dw", [1, 1], F32, kind="ExternalInput")
    dp_d = nc.dram_tensor("dp", [1, 1], F32, kind="ExternalInput")
    o_d = nc.dram_tensor("o", [LC, D], F32, kind="ExternalOutput")

    q_r = q_d.rearrange("(n p) d -> p n d", p=P)
    k_r = k_d.rearrange("(n p) d -> p n d", p=P)
    v_r = v_d.rearrange("(n p) d -> p n d", p=P)
    o_r = o_d.rearrange("(n p) d -> p n d", p=P)

    from contextlib import ExitStack

    with tile.TileContext(nc) as tc, ExitStack() as ctx:
        ek = ctx.enter_context
        sing = ek(tc.tile_pool(name="sing", bufs=1))
        pqn = ek(tc.tile_pool(name="qn", bufs=4))        # [128,512] f32 wave
        pkn = ek(tc.tile_pool(name="kn", bufs=8))
        pvn = ek(tc.tile_pool(name="vn", bufs=4))
        ptn = ek(tc.tile_pool(name="tn", bufs=4))
        ptk = ek(tc.tile_pool(name="tnk", bufs=NST))        # transformed nat bf16
        pqt = ek(tc.tile_pool(name="qt", bufs=NHP))      # tqT/tkT per pair
        pqts = ek(tc.tile_pool(name="qts", bufs=2))
        pmb = ek(tc.tile_pool(name="mb", bufs=2))
        pat = ek(tc.tile_pool(name="at", bufs=2))        # A^T per head bf16
        pva = ek(tc.tile_pool(name="va", bufs=4))        # Vaug, 2 pairs in flight
        pot = ek(tc.tile_pool(name="ot", bufs=1))
        psc = ek(tc.tile_pool(name="small", bufs=4))
        pvw = ek(tc.tile_pool(name="varw", bufs=10))     # var-wave [128,64] tiles
        pst = ek(tc.tile_pool(name="stat", bufs=2))      # [128,512] f32 scratch
        pgw = ek(tc.tile_pool(name="gw", bufs=1))        # Gsb/Wsb/prod
        prw = ek(tc.tile_pool(name="rows", bufs=1))      # [8,1024] m-chain
        pdr = ek(tc.tile_pool(name="dr", bufs=2, space="DRAM"))
        pps = ek(tc.tile_pool(name="ps", bufs=2, space="PSUM"))    # [128,1024]
        ppb = ek(tc.tile_pool(name="psb", bufs=1, space="PSUM"))   # G/rows/po
        ppt = ek(tc.tile_pool(name="ptr", bufs=2, space="PSUM"))   # transposes

        # --- constants ---
        ident = sing.tile([P, P], BF16)
        make_identity(nc, ident)
        zero_t = sing.tile([P, 1], F32)
        nc.vector.memset(zero_t, 0.0)
        eps_t = sing.tile([P, 1], F32)
        nc.vector.memset(eps_t, EPS)
        dw_t = sing.tile([P, 1], F32)
        nc.sync.dma_start(out=dw_t, in_=dw_d[:, :].to_broadcast([P, 1]))
        dp_t = sing.tile([P, 1], F32)
        nc.sync.dma_start(out=dp_t, in_=dp_d[:, :].to_broadcast([P, 1]))
        dp2 = sing.tile([P, 1], F32)
        nc.vector.tensor_mul(dp2, dp_t, dp_t)
        c2 = sing.tile([P, 1], F32)  # scale * dyn^2
        nc.vector.tensor_scalar_mul(c2, dp2, float(SCALE))
        dp4 = sing.tile([P, 1], F32)
        nc.vector.tensor_mul(dp4, dp2, dp2)
        a_t = sing.tile([P, 1], F32)  # dyn^4 * UNB_S / S      (sumsq coeff)
        nc.vector.tensor_scalar_mul(a_t, dp4, UNB_S / S)
        b_t = sing.tile([P, 1], F32)  # dyn^4 * UNB_S / S^2    (mean^2 coeff)
        nc.vector.tensor_scalar_mul(b_t, dp4, UNB_S / S / S)
        ones1 = sing.tile([P, 1], BF16)
        nc.vector.memset(ones1, 1.0)
        ones2 = sing.tile([P, 2], BF16)  # block ones for per-head column sums
        nc.vector.memset(ones2, 0.0)
        nc.vector.memset(ones2[0:E, 0:1], 1.0)
        nc.vector.memset(ones2[E:P, 1:2], 1.0)

        # --- transform waves: load nat chunks, headwise var on DVE, then
        #     batched sqrt / tanh on ACT (minimal act-table swaps) ---
        def transform_wave(src_r, i0, n, ttag, out_list):
            nats, vars_, chunks = [], [], []
            for i in range(i0, i0 + n):
                if ttag == "tqn":
                    nat = pqn.tile([P, D], F32, tag="qn")
                else:
                    nat = pkn.tile([P, D], F32, tag="kn")
                nc.sync.dma_start(out=nat, in_=src_r[:, i, :])
                sq = pst.tile([P, D], F32, tag="sq")
                nc.gpsimd.tensor_mul(sq, nat, nat)
                ssum = pvw.tile([P, E], F32, tag="ssum")
                ssq = pvw.tile([P, E], F32, tag="ssq")
                nc.vector.tensor_reduce(ssum, nat.rearrange("p (h e) -> p e h", h=H),
                                        axis=mybir.AxisListType.X,
                                        op=mybir.AluOpType.add)
                nc.vector.tensor_reduce(ssq, sq.rearrange("p (h e) -> p e h", h=H),
                                        axis=mybir.AxisListType.X,
                                        op=mybir.AluOpType.add)
                nc.vector.tensor_scalar_mul(ssum, ssum, 1.0 / H)   # mean
                nc.vector.tensor_mul(ssum, ssum, ssum)             # mean^2
                nc.vector.tensor_scalar_mul(ssq, ssq, 1.0 / H)
                nc.vector.tensor_sub(ssq, ssq, ssum)               # biased var
                nats.append(nat)
                vars_.append(ssq)
                chunks.append(i)
            for ssq in vars_:  # batched sqrt (one act table)
                nc.scalar.activation(ssq, ssq, AF.Sqrt, bias=zero_t, scale=UNB_H)
            for ssq in vars_:
                nc.vector.tensor_scalar_add(ssq, ssq, EPS)
                nc.vector.reciprocal(ssq, ssq)                     # rstd
            for nat, ssq, i in zip(nats, vars_, chunks):
                tmp = pst.tile([P, D], F32, tag="tmp")
                nc.vector.tensor_mul(tmp, nat, _head_bcast(ssq[:, :]))
                if ttag == "tqn":
                    tn = ptn.tile([P, D], BF16, tag=ttag)
                else:
                    tn = ptk.tile([P, D], BF16, tag=ttag)
                nc.scalar.activation(tn, tmp, AF.Tanh, bias=zero_t, scale=dw_t)
                out_list.append((i, tn))

        tq_chunks, tk_chunks = [], []
        transform_wave(q_r, 0, 4, "tqn", tq_chunks)
        transform_wave(q_r, 4, 4, "tqn", tq_chunks)
        transform_wave(k_r, 0, 8, "tkn", tk_chunks)
        transform_wave(k_r, 8, 8, "tkn", tk_chunks)

        # --- transpose into packed [2E, l] per pair; Gram matrix for k ---
        tqT = []
        tkT = []
        for _hp in range(NHP):
            qT_t = pqt.tile([P, LC], BF16, tag="tqT")
            tqT.append(qT_t)
            kT_t = pqt.tile([P, S], BF16, tag="tkT")
            tkT.append(kT_t)
        for chunks, dsts in ((tq_chunks, tqT), (tk_chunks, tkT)):
            for j in range(0, len(chunks), 2):
                for hp in range(NHP):
                    pt = ppt.tile([P, 2, P], BF16, tag="tp")
                    for u in range(2):
                        _i, tn = chunks[j + u]
                        nc.tensor.transpose(pt[:, u, :],
                                            tn[:, hp * P : (hp + 1) * P], ident)
                    i0 = chunks[j][0]
                    nc.vector.tensor_copy(
                        dsts[hp][:, i0 * P : (i0 + 2) * P],
                        pt.rearrange("p a b -> p (a b)"),
                    )

        # --- per-pair: block-diag G, ksum; row-stats -> m for all heads ---
        gsb = []
        k2s = []
        for hp in range(NHP):
            g_ps = ppb.tile([P, P], F32, tag="pb")
            for idx, (i, tn) in enumerate(tk_chunks):
                nc.tensor.matmul(g_ps, tn[:, hp * P : (hp + 1) * P],
                                 tn[:, hp * P : (hp + 1) * P],
                                 start=(idx == 0), stop=(idx == NST - 1))
            g = pgw.tile([P, P], BF16, tag="gsb")
            nc.vector.tensor_copy(g, g_ps)
            nc.vector.memset(g[0:E, E:P], 0.0)   # zero cross-head blocks
            nc.vector.memset(g[E:P, 0:E], 0.0)
            gsb.append(g)
            ks_ps = ppb.tile([P, 1], F32, tag="pb")
            for idx, (i, tn) in enumerate(tk_chunks):
                nc.tensor.matmul(ks_ps, tn[:, hp * P : (hp + 1) * P], ones1,
                                 start=(idx == 0), stop=(idx == NST - 1))
            ks = pvw.tile([P, 1], F32, tag="ks")
            nc.vector.tensor_copy(ks, ks_ps)
            k2 = pgw.tile([P, 2], BF16, tag="k2")
            nc.vector.memset(k2, 0.0)
            nc.vector.tensor_copy(k2[0:E, 0:1], ks[0:E, :])
            nc.vector.tensor_copy(k2[E:P, 1:2], ks[E:P, :])
            k2s.append(k2)

        ssq_sb = prw.tile([8, LC], F32, tag="ssqsb")
        rsum_sb = prw.tile([8, LC], F32, tag="rsumsb")
        for hp in range(NHP):
            wps = pps.tile([P, LC], F32, tag="ps")
            for j in range(2):
                nc.tensor.matmul(wps[:, j * 512 : (j + 1) * 512], gsb[hp],
                                 tqT[hp][:, j * 512 : (j + 1) * 512],
                                 start=True, stop=True)
            wsb = pgw.tile([P, LC], BF16, tag="wsb")
            nc.scalar.copy(wsb, wps)
            prod = pgw.tile([P, LC], BF16, tag="prod")
            nc.vector.tensor_mul(prod, tqT[hp], wsb)
            rows_ss = ppb.tile([2, LC], F32, tag="pb")
            rows_rs = ppb.tile([2, LC], F32, tag="pb")
            for j in range(2):
                nc.tensor.matmul(rows_ss[:, j * 512 : (j + 1) * 512], ones2,
                                 prod[:, j * 512 : (j + 1) * 512],
                                 start=True, stop=True)
                nc.tensor.matmul(rows_rs[:, j * 512 : (j + 1) * 512], k2s[hp],
                                 tqT[hp][:, j * 512 : (j + 1) * 512],
                                 start=True, stop=True)
            stg_ss = pgw.tile([2, LC], F32, tag="stgss")
            nc.vector.tensor_copy(stg_ss, rows_ss)
            nc.sync.dma_start(out=ssq_sb[2 * hp : 2 * hp + 2, :], in_=stg_ss)
            stg_rs = pgw.tile([2, LC], F32, tag="stgrs")
            nc.vector.tensor_copy(stg_rs, rows_rs)
            nc.sync.dma_start(out=rsum_sb[2 * hp : 2 * hp + 2, :], in_=stg_rs)

        # m = c2 / sqrt(ssq*a - rsum^2*b + eps), vectorized over 8 heads
        nc.vector.tensor_mul(rsum_sb, rsum_sb, rsum_sb)
        nc.vector.tensor_scalar_mul(rsum_sb, rsum_sb, b_t[0:8, :])
        nc.vector.tensor_scalar_mul(ssq_sb, ssq_sb, a_t[0:8, :])
        nc.vector.tensor_sub(ssq_sb, ssq_sb, rsum_sb)
        nc.scalar.activation(ssq_sb, ssq_sb, AF.Sqrt, bias=eps_t[0:8, :], scale=1.0)
        nc.vector.reciprocal(ssq_sb, ssq_sb)
        nc.vector.tensor_scalar_mul(ssq_sb, ssq_sb, c2[0:8, :])
        m8b = prw.tile([8, LC], BF16, tag="m8b")
        nc.vector.tensor_copy(m8b, ssq_sb)
        mdr = pdr.tile([8, LC], BF16, tag="mdr")
        nc.sync.dma_start(out=mdr[:, :], in_=m8b)

        # qts per pair: tq * m (broadcast m rows from DRAM)
        qts_l = []
        for hp in range(NHP):
            mb = pmb.tile([P, LC], BF16, tag="mb")
            for local in range(2):
                h = 2 * hp + local
                nc.sync.dma_start(out=mb[local * E : (local + 1) * E, :],
                                  in_=mdr[h : h + 1, :].to_broadcast([E, LC]))
            qts = pqts.tile([P, LC], BF16, tag="qts")
            nc.vector.tensor_mul(qts, tqT[hp], mb)
            qts_l.append(qts)

        # --- st -> exp -> A^T (head h), then flipped A^T@Vaug of head h-1 ---
        vas_by_head = {}

        def emit_vaug_pair(hp):
            for local in range(2):
                va = pva.tile([P, NST, E + 1], BF16, tag="va")
                nc.vector.memset(va[:, :, E : E + 1], 1.0)
                vas_by_head[2 * hp + local] = va
            for kk in range(NST):
                vn = pvn.tile([P, D], F32, tag="vn")
                nc.sync.dma_start(out=vn, in_=v_r[:, kk, :])
                for local in range(2):
                    h = 2 * hp + local
                    nc.gpsimd.tensor_copy(vas_by_head[h][:, kk, 0:E],
                                          vn[:, h * E : (h + 1) * E])

        def emit_st(h):
            hp, local = h // 2, h % 2
            off = local * E
            tk = tkT[hp]
            at = pat.tile([P, NST, LC], BF16, tag="at")
            for kk in range(NST):
                st_ps = pps.tile([P, LC], F32, tag="ps")
                for lb in range(NLB):
                    nc.tensor.matmul(
                        st_ps[:, lb * LB : (lb + 1) * LB],
                        tk[off : off + E, kk * P : (kk + 1) * P],
                        qts_l[hp][off : off + E, lb * LB : (lb + 1) * LB],
                        start=True, stop=True,
                    )
                nc.scalar.activation(at[:, kk, :], st_ps, AF.Exp,
                                     bias=zero_t, scale=1.0)
            return at

        def emit_av(h, at):
            va = vas_by_head[h]
            for lt in range(LC // P):
                po = ppb.tile([P, E + 1], F32, tag="pb")
                for kk in range(NST):
                    nc.tensor.matmul(po, at[:, kk, lt * P : (lt + 1) * P],
                                     va[:, kk, :],
                                     start=(kk == 0), stop=(kk == NST - 1))
                rec = psc.tile([P, 1], F32, tag="rec")
                nc.vector.reciprocal(rec, po[:, E : E + 1])
                ob = psc.tile([P, 1, E], F32, tag="ob")
                nc.vector.tensor_scalar_mul(ob[:, 0, :], po[:, 0:E], rec)
                nc.sync.dma_start(out=o_r[:, lt : lt + 1, h * E : (h + 1) * E],
                                  in_=ob)

        prev = None
        for h in range(H):
            if h % 2 == 0:
                emit_vaug_pair(h // 2)
            at = emit_st(h)
            if prev is not None:
                emit_av(*prev)
            prev = (h, at)
        emit_av(*prev)

    return nc


_nc_cache = None


def kernel(queries, keys, values, attn_mask=None, directional_weights=None,
           dynamic_param=None, **_unused):
    global _nc_cache, _last_exec_time_ns
    q = np.asarray(queries, dtype=np.float32)
    k = np.asarray(keys, dtype=np.float32)
    v = np.asarray(values, dtype=np.float32)
    dw = np.asarray(directional_weights, dtype=np.float32).reshape(1, 1)
    dp = np.asarray(dynamic_param, dtype=np.float32).reshape(1, 1)

    if _nc_cache is None:
        nc = build_nc()
        nc.finalize()
        _nc_cache = nc
    nc = _nc_cache

    in_maps = []
    for c in range(8):
        b, lh = c // 2, c % 2
        in_maps.append({
            "q": np.ascontiguousarray(q[b, lh * LC : (lh + 1) * LC]).reshape(LC, D),
            "k": np.ascontiguousarray(k[b]).reshape(S, D),
            "v": np.ascontiguousarray(v[b]).reshape(S, D),
            "dw": dw, "dp": dp,
        })

    tracing = bool(os.environ.get("BASS_TRACE"))
    if tracing:
        _ensure_axon_hooks()
        import concourse.bass_utils as _bu

        _orig_upload = _bu.upload_artifacts
        _bu.upload_artifacts = lambda d: d  # no bucket access in this sandbox
        try:
            res = run_bass_kernel_spmd(nc, in_maps, core_ids=list(range(8)))
        except Exception as e:  # fall back to an untraced run
            print(f"traced run failed ({e!r}); retrying untraced", file=sys.stderr)
            os.environ["BASS_NEVER_TRACE"] = "1"
            try:
                res = run_bass_kernel_spmd(nc, in_maps, core_ids=list(range(8)))
            finally:
                os.environ.pop("BASS_NEVER_TRACE", None)
        finally:
            _bu.upload_artifacts = _orig_upload
    else:
        res = run_bass_kernel_spmd(nc, in_maps, core_ids=list(range(8)))
    _last_exec_time_ns = res.exec_time_ns

    out = np.empty((B, L, H, E), dtype=np.float32)
    for c in range(8):
        b, lh = c // 2, c % 2
        out[b, lh * LC : (lh + 1) * LC] = res.results[c]["o"].reshape(LC, H, E)
    return out

